# revision 40
# baseline (speedup 1.0000x reference)
"""Trainium2 Bass kernel for an Adapter block (LN -> 768x64 -> ReLU -> 64x768).

Data-parallel over the batch dim (8 batches -> 8 NeuronCores). Per core:
x shard [4096, 768], processed in 8 groups of 512 tokens.

Math (lazy LayerNorm, folded into the matmuls):
  LN(x) = (x - mu) * r * gamma + beta,   r = rsqrt(var + eps)
  down  = LN(x) @ W_d + b_d = r * (x @ Wg - mu*sg) + c,   Wg = diag(gamma) W_d
  Since r > 0:  relu(down) = r * relu(x@Wg - mu (x) sg + sd (x) c),  sd = 1/r
  out   = relu(down) @ W_u + b_u = r * (relu_z @ W_u) + b_u

Kernel structure per 512-token group:
  - down-proj runs with Wg STATIONARY (65 cols: 64 wg + one mu-column of
    1/768), streaming raw bf16 x chunks (N=512 moving) -> psum [65, 512]
    with k on partitions and mean in a psum row.
  - E[x^2] row via a (1/768)-stationary reduce over x*x (squares on DVE).
  - the "- mu (x) sg [+ sd (x) c]" fixup is ONE rank-1(2) accumulate matmul
    into the same psum (lhsT = [-sg; c] rows, rhs = [mu; sd] rows).
  - groups alternate psum row placement (A: rows 0-64, B: rows 63-127) so
    consecutive groups' up-proj matmuls (contract=64) occupy disjoint PE
    row groups and run concurrently.
  - r is applied per-partition (token) during the psum->sbuf output copy
    (ACT activation with scale=r column / DVE tensor_scalar), output bf16.

I/O: x is shipped pre-transposed/pre-tiled bf16 [128, 8, 6, 512] (host
cast+relayout is free); output is bf16 [4096, 768], host upcasts to f32.
"""

import numpy as np

D_MODEL = 768
BOTTLENECK = 64
LN_EPS = 1e-5
SCALE = 1.0
N_CORES = 8
TOK = 4096
P = 128
NCH = D_MODEL // P   # 6 feature chunks
NG = 8               # token groups per core
TG = TOK // NG       # 512 tokens per group
NTJ = TG // P        # 4 token tiles per group
K = BOTTLENECK
INV_D = 1.0 / D_MODEL

_CACHE = {}


def _build(general):
    import concourse.bacc as bacc
    import concourse.bass as bass
    import concourse.tile as tile
    from concourse import mybir
    from concourse.masks import make_identity
    from contextlib import ExitStack

    f32 = mybir.dt.float32
    f32r = mybir.dt.float32r
    bf16 = mybir.dt.bfloat16
    AF = mybir.ActivationFunctionType
    OP = mybir.AluOpType

    nc = bacc.Bacc("TRN2", target_bir_lowering=False, debug=False,
                   num_devices=N_CORES)

    x_d = nc.dram_tensor("x", [P, NG, NCH, TG], bf16, kind="ExternalInput").ap()
    wga_d = nc.dram_tensor("wga", [P, NCH, K + 2], bf16, kind="ExternalInput").ap()
    wua_d = nc.dram_tensor("wua", [2 * K, D_MODEL], bf16, kind="ExternalInput").ap()
    lt_d = nc.dram_tensor("lt", [2, K], bf16, kind="ExternalInput").ap()
    if general:
        bup_d = nc.dram_tensor("bup", [D_MODEL], f32, kind="ExternalInput").ap()
    out_d = nc.dram_tensor("out", [TOK, D_MODEL], bf16, kind="ExternalOutput").ap()
    out_r = out_d.rearrange("(g j p) d -> p g j d", g=NG, j=NTJ, p=P)

    with tile.TileContext(nc, pool_alloc_mode="queue") as tc, ExitStack() as ctx:
        consts = ctx.enter_context(tc.tile_pool(name="consts", bufs=1))
        xg_pool = ctx.enter_context(tc.tile_pool(name="xg", bufs=4))
        sq_pool = ctx.enter_context(tc.tile_pool(name="sq", bufs=2))
        row_pool = ctx.enter_context(tc.tile_pool(name="rows", bufs=3))
        st_pool = ctx.enter_context(tc.tile_pool(name="stats", bufs=3))
        relu_pool = ctx.enter_context(tc.tile_pool(name="relu", bufs=3))
        out_pool = ctx.enter_context(tc.tile_pool(name="outp", bufs=3))
        ps_down = ctx.enter_context(
            tc.tile_pool(name="ps_down", bufs=2 if general else 3, space="PSUM"))
        ps_up = ctx.enter_context(tc.tile_pool(name="ps_up", bufs=2, space="PSUM"))
        ps_tiny = ctx.enter_context(tc.tile_pool(name="ps_tiny", bufs=1, space="PSUM"))
        ps_gen = (ctx.enter_context(tc.tile_pool(name="ps_gen", bufs=1, space="PSUM"))
                  if general else None)

        # ---------------- constants ----------------
        # group-0 input DMA issued before anything else
        xgs = {}
        xq0 = xg_pool.tile([P, NCH, TG], bf16, tag="xg")
        nc.sync.dma_start(out=xq0, in_=x_d[:, 0])
        xgs[0] = xq0
        idb = consts.tile([P, P], bf16)
        make_identity(nc, idb)
        wga_sb = consts.tile([P, NCH, K + 2], bf16)
        nc.sync.dma_start(out=wga_sb, in_=wga_d)
        wua_sb = consts.tile([2 * K, D_MODEL], bf16)
        nc.sync.dma_start(out=wua_sb, in_=wua_d)
        # rank-1(2) lhsT rows: [-sg; c] at partitions 64:66
        lt_sb = consts.tile([P, K], bf16)
        nc.sync.dma_start(out=lt_sb[64:66, :], in_=lt_d)
        # E[x^2] reduce stationary: col 1 = 1/768, rest 0
        red_sb = consts.tile([P, 2], bf16)
        nc.vector.memset(red_sb, 0.0)
        nc.vector.memset(red_sb[:, 1:2], INV_D)
        eps_t = consts.tile([P, 1], f32)
        nc.vector.memset(eps_t, LN_EPS)
        # pre-issue the next input DMAs after the consts
        for g0 in (1, 2, 3):
            xq = xg_pool.tile([P, NCH, TG], bf16, tag="xg")
            nc.sync.dma_start(out=xq, in_=x_d[:, g0])
            xgs[g0] = xq
        if not general:
            # keep the PE busy while group-0 input streams in: un-throttles
            # the HAM clock gate before the real matmuls start. The warm
            # tile is a ps_down rotation slot, reused by group 0's psum.
            wps = ps_down.tile([P, TG], f32, tag="dps")
            for _ in range(44):
                nc.tensor.matmul(wps[0:1, 0:P], lhsT=idb[:, 0:1], rhs=idb,
                                 start=True, stop=True)
        if general:
            bupb = consts.tile([P, D_MODEL], bf16)
            nc.gpsimd.dma_start(
                out=bupb,
                in_=bass.AP(tensor=bup_d.tensor, offset=bup_d.offset,
                            ap=[[0, P], [1, D_MODEL]]))

        state = {}

        def front(g, slots=None):
            # psum row layout: wg rows 0-63, mu row 64, E[x^2] row 65
            d_sl = slice(0, K + 2)
            s2_sl = slice(K, K + 2)
            red = red_sb
            r_sl = slice(0, K)
            rows_sl = slice(K, K + 2)
            wg_sb = wga_sb

            def pop():
                if slots:
                    slots.popleft()()

            if g in xgs:
                xg = xgs.pop(g)
            else:
                xg = xg_pool.tile([P, NCH, TG], bf16, tag="xg")
                nc.sync.dma_start(out=xg, in_=x_d[:, g])

            sq = sq_pool.tile([P, NCH, TG], bf16)
            nc.vector.tensor_tensor(out=sq, in0=xg, in1=xg, op=OP.mult)

            dps = ps_down.tile([P, TG], f32, tag="dps")
            for c in range(NCH):
                nc.tensor.matmul(dps[d_sl, :], lhsT=wg_sb[:, c, :],
                                 rhs=xg[:, c, :],
                                 start=(c == 0), stop=(c == NCH - 1))
                if c in (2, 5):
                    pop()
            for c in range(NCH):
                nc.tensor.matmul(dps[s2_sl, :], lhsT=red,
                                 rhs=sq[:, c, :],
                                 start=False, stop=(c == NCH - 1),
                                 skip_group_check=True)
                if c in (2, 5):
                    pop()

            # stat rows [mu; ex2] -> sbuf (bf16), same partitions
            rows12 = row_pool.tile([P, TG], bf16)
            nc.scalar.activation(out=rows12[rows_sl, :], in_=dps[rows_sl, :],
                                 func=AF.Copy)

            # transpose stat rows -> columns (psum), interleaved [mu, ex2]
            stc = ps_tiny.tile([P, NTJ, 2], bf16)
            for j in range(NTJ):
                nc.tensor.transpose(stc[:, j:j + 1, :],
                                    rows12[rows_sl, j * P:(j + 1) * P],
                                    idb[rows_sl, rows_sl])

            # stats math on columns: var = ex2 - mu^2 ; r = rsqrt(var+eps)
            stcs = st_pool.tile([P, NTJ, 2], f32, tag="stcs")
            nc.vector.tensor_copy(out=stcs, in_=stc)
            s1c = stcs[:, :, 0:1]
            s2c = stcs[:, :, 1:2]
            t1 = st_pool.tile([P, NTJ, 1], f32, tag="t1")
            nc.scalar.activation(out=t1, in_=s1c, func=AF.Square)
            u = st_pool.tile([P, NTJ, 1], f32, tag="u")
            nc.vector.tensor_tensor(out=u, in0=s2c, in1=t1, op=OP.subtract)
            sdc = st_pool.tile([P, NTJ, 1], f32, tag="sd")
            nc.scalar.activation(out=sdc, in_=u, func=AF.Sqrt, bias=eps_t,
                                 scale=1.0)
            rc = st_pool.tile([P, NTJ, 1], f32, tag="rc")
            nc.vector.reciprocal(out=rc, in_=sdc)
            pop()

            if general:
                # need sd = sqrt(var+eps) as the second stat ROW for the
                # c (x) sd term: transpose [mu; sd] columns back into rows.
                msd = st_pool.tile([P, NTJ, 2], bf16, tag="msd")
                nc.vector.tensor_copy(out=msd[:, :, 0:1], in_=s1c)
                nc.vector.tensor_copy(out=msd[:, :, 1:2], in_=sdc)
                sd_ps = ps_gen.tile([P, TG], bf16)
                for j in range(NTJ):
                    nc.tensor.transpose(sd_ps[rows_sl, j * P:(j + 1) * P],
                                        msd[:, j, :], idb)
                nc.scalar.activation(out=rows12[rows_sl, :],
                                     in_=sd_ps[rows_sl, :], func=AF.Copy)

            # rank-1(2) fixup accumulate: psum[wg rows] += [-sg; c].T @ [mu; sd]
            nc.tensor.matmul(dps[r_sl, :], lhsT=lt_sb[rows_sl, :],
                             rhs=rows12[rows_sl, :],
                             start=False, stop=True, skip_group_check=True)

            # relu -> bf16 sbuf (same partition half)
            relu_t = relu_pool.tile([P, TG], bf16)
            nc.scalar.activation(out=relu_t[r_sl, :], in_=dps[r_sl, :],
                                 func=AF.Relu)
            if g % 2 == 1:
                # odd groups: shift relu to partitions 64-127 so the paired
                # up-proj matmuls occupy the other PE row-tile (T8) and run
                # concurrently with the even group's (T0)
                nc.gpsimd.dma_start(out=relu_t[K:P, :], in_=relu_t[0:K, :])
            pop()

            state[g] = (relu_t, rc)

        def queue_pair(pair, slots):
            ga, gb = 2 * pair, 2 * pair + 1
            relu_a, rc_a = state.pop(ga)
            relu_b, rc_b = state.pop(gb)
            oga = out_pool.tile([P, NTJ, D_MODEL], bf16, tag="og")
            ogb = out_pool.tile([P, NTJ, D_MODEL], bf16, tag="og")
            outs = {ga: oga, gb: ogb}
            rcs = {ga: rc_a, gb: rc_b}
            relus = {ga: relu_a, gb: relu_b}

            def up_set(j):

                def emit():
                    u8s = {}
                    for g, hs in ((ga, slice(0, K)), (gb, slice(K, P))):
                        lhs = relus[g][hs, j * P:(j + 1) * P]
                        u8 = ps_up.tile([P, D_MODEL], f32, tag="u8")
                        u8s[g] = u8
                        nc.tensor.matmul(u8[:, 0:512], lhsT=lhs,
                                         rhs=wua_sb[hs, 0:512],
                                         start=True, stop=True)
                    for g, hs in ((ga, slice(0, K)), (gb, slice(K, P))):
                        lhs = relus[g][hs, j * P:(j + 1) * P]
                        nc.tensor.matmul(u8s[g][:, 512:768], lhsT=lhs,
                                         rhs=wua_sb[hs, 512:768],
                                         start=True, stop=True)
                    for gi, g in enumerate((ga, gb)):
                        og = outs[g]
                        rcj = rcs[g][:, j:j + 1, :]
                        if gi == 0:
                            nc.scalar.activation(out=og[:, j, :], in_=u8s[g],
                                                 func=AF.Copy, bias=0.0,
                                                 scale=rcj)
                        else:
                            nc.vector.tensor_scalar(out=og[:, j, :],
                                                    in0=u8s[g],
                                                    scalar1=rcj, scalar2=None,
                                                    op0=OP.mult)
                        if general:
                            nc.vector.tensor_tensor(out=og[:, j, :],
                                                    in0=og[:, j, :],
                                                    in1=bupb, op=OP.add)
                return emit

            def ship(lo, hi):
                def emit():
                    for g in (ga, gb):
                        nc.sync.dma_start(out=out_r[:, g, lo:hi],
                                          in_=outs[g][:, lo:hi, :])
                return emit

            slots.append(up_set(0))
            slots.append(up_set(1))
            slots.append(ship(0, 2))
            slots.append(up_set(2))
            slots.append(up_set(3))
            slots.append(ship(2, 4))

        # software pipeline: up-proj work of pair p is interleaved into the
        # down-matmul stream of groups 2p+2 / 2p+3 so the PE never stalls on
        # psum drains
        from collections import deque
        slots = deque()
        for g in range(NG):
            front(g, slots)
            if g % 2 == 1:
                queue_pair(g // 2, slots)
        while slots:
            slots.popleft()()

    nc.compile()
    return nc


def _get_nc(general):
    key = ("nc", general)
    if key not in _CACHE:
        _CACHE[key] = _build(general)
    return _CACHE[key]


def _in_maps(x, ln_gamma, ln_beta, w_down, b_down, w_up, b_up):
    import ml_dtypes
    f = np.float32
    bf = ml_dtypes.bfloat16
    x = np.asarray(x, dtype=f)
    ln_gamma = np.asarray(ln_gamma, dtype=f)
    ln_beta = np.asarray(ln_beta, dtype=f)
    w_down = np.asarray(w_down, dtype=f)
    b_down = np.asarray(b_down, dtype=f)
    w_up = np.asarray(w_up, dtype=f)
    b_up = np.asarray(b_up, dtype=f)

    wg = ln_gamma[:, None] * w_down                      # [768, 64]
    sg = wg.sum(axis=0)                                  # [64]
    cc = ln_beta @ w_down + b_down                       # [64]
    general = bool(np.any(b_up)) or bool(np.any(cc))

    # stationary block: [p, c, 66] with f = c*128 + p
    wg_pc = wg.reshape(NCH, P, K).transpose(1, 0, 2)     # [p, c, k]
    wga = np.zeros((P, NCH, K + 2), f)
    wga[:, :, 0:K] = wg_pc
    wga[:, :, K] = INV_D                                 # mu column
    # col K+1 stays 0 (E[x^2] row target)

    # rank-1(2) lhsT rows (partitions 64, 65): [-sg, c]
    lt = np.stack([-sg, cc], axis=0)                     # [2, 64]

    common = {
        "wga": np.ascontiguousarray(wga.astype(bf)),
        "wua": np.ascontiguousarray(np.concatenate([w_up, w_up], 0).astype(bf)),
        "lt": np.ascontiguousarray(lt.astype(bf)),
    }
    if general:
        common["bup"] = np.ascontiguousarray(b_up)

    maps = []
    for i in range(N_CORES):
        xt = x[i].T                                      # [768, 4096]
        xr = xt.reshape(NCH, P, NG, TG).transpose(1, 2, 0, 3)  # [p, g, c, t]
        maps.append(dict(common, x=np.ascontiguousarray(xr.astype(bf))))
    return general, maps


def run(trace=False, **inputs):
    """Run the SPMD kernel; returns (output, BassKernelResults)."""
    from concourse.bass_utils import run_bass_kernel_spmd
    general, in_maps = _in_maps(**inputs)
    nc = _get_nc(general)
    res = run_bass_kernel_spmd(nc, in_maps, core_ids=list(range(N_CORES)),
                               trace=trace)
    out = np.stack([res.results[i]["out"].astype(np.float32)
                    for i in range(N_CORES)], axis=0)
    return out, res


def kernel(**inputs) -> np.ndarray:
    out, _ = run(trace=False, **inputs)
    return out
